# revision 6
# baseline (speedup 1.0000x reference)
"""Diagonal Mahalanobis distance kernel for Trainium2 (8 NeuronCores, SPMD).

d2[n, m] = sum_d (s_d * (x[n,d] - y[m,d]))^2
         = ||xs_n||^2 + ||ys_m||^2 - 2 * xs @ ys^T,   xs = x*s, ys = y*s, s = exp(log_scale)

Sharding: x rows (and output rows) split across 8 cores; y and log_scale replicated.
Per core: x_shard (1024, 512) -> out_shard (1024, 8192).

Per-core algorithm:
  - s_part [128, 4]: exp(log_scale) with D on partitions (for scaling transposed tiles)
  - s_rep  [128, 512]: exp(log_scale) replicated across partitions (ones-matmul trick)
  - xs = x * s_rep; xn[i] = row sums of xs^2 via ACT Square+accum; xsT via PE transpose
  - per 512-col j-chunk: transpose y (PE), scale by s_part in the PSUM->SBUF copyback,
    square (ACT) + ones-matmul -> yn replicated across partitions [128, 512],
    GEMM xsT.T @ ysT in float32r (1 cyc/row vs 4 for fp32),
    epilogue: ACT (-2*psum + xn[i]) then DVE (+ yn_rep), DMA out.
"""

import os
from contextlib import ExitStack

import numpy as np

import concourse.bass as bass
import concourse.tile as tile
from concourse import bacc, mybir
from concourse.bass import ds, ts
from concourse.bass_utils import run_bass_kernel_spmd
from concourse.masks import make_identity

N, M, D = 8192, 8192, 512
NCORES = 8
NS = N // NCORES  # 1024 x-rows per core
P = 128
KC = D // P       # 4 contraction chunks of 128
NIT = NS // P     # 8 i-tiles per core
JBLK = 512
NJ = M // JBLK    # 16 j-chunks
JSUB = JBLK // P  # 4

F32 = mybir.dt.float32
F32R = mybir.dt.float32r
AF = mybir.ActivationFunctionType


def _build_program():
    nc = bacc.Bacc("TRN2", target_bir_lowering=False, debug=False)

    x_d = nc.dram_tensor("x", [NS, D], F32, kind="ExternalInput").ap()
    y_d = nc.dram_tensor("y", [M, D], F32, kind="ExternalInput").ap()
    ls_d = nc.dram_tensor("log_scale", [D], F32, kind="ExternalInput").ap()
    out_d = nc.dram_tensor("out", [NS, M], F32, kind="ExternalOutput").ap()

    with tile.TileContext(nc) as tc, ExitStack() as ctx:
        consts = ctx.enter_context(tc.tile_pool(name="consts", bufs=1))
        xpool = ctx.enter_context(tc.tile_pool(name="xpool", bufs=2))
        ypool = ctx.enter_context(tc.tile_pool(name="ypool", bufs=3))
        ytp = ctx.enter_context(tc.tile_pool(name="ytp", bufs=2))
        opool = ctx.enter_context(tc.tile_pool(name="opool", bufs=6))
        tp_ps = ctx.enter_context(tc.tile_pool(name="tp_ps", bufs=2, space="PSUM"))
        yn_ps = ctx.enter_context(tc.tile_pool(name="yn_ps", bufs=2, space="PSUM"))
        mm_ps = ctx.enter_context(tc.tile_pool(name="mm_ps", bufs=4, space="PSUM"))

        ident = consts.tile([P, P], F32)
        make_identity(nc, ident)
        ones = consts.tile([P, P], F32)
        nc.vector.memset(ones, 1.0)
        ones_r = consts.tile([P, P], F32R)
        nc.vector.tensor_copy(ones_r, ones)

        # --- scale vectors ---
        # s_part[p, k] = exp(log_scale[k*128 + p]) (D on partitions)
        ls_part = consts.tile([P, KC], F32)
        nc.sync.dma_start(ls_part, ls_d.rearrange("(o p) -> p o", p=P))
        s_part = consts.tile([P, KC], F32)
        nc.scalar.activation(s_part, ls_part, AF.Exp)

        # s_rep[p, d] = exp(log_scale[d]) for every partition p
        ls_row = consts.tile([1, D], F32)
        nc.sync.dma_start(ls_row, ls_d[None, :])
        s_row = consts.tile([1, D], F32)
        nc.scalar.activation(s_row, ls_row, AF.Exp)
        ps_srep = yn_ps.tile([P, D], F32, tag="yn")
        nc.tensor.matmul(ps_srep, ones[0:1, :], s_row, start=True, stop=True)
        s_rep = consts.tile([P, D], F32)
        nc.vector.tensor_copy(s_rep, ps_srep)

        # --- x phase: xs, xn, xsT ---
        xn_all = consts.tile([P, NIT], F32)  # xn per i-tile, i on partitions
        xsT = [consts.tile([P, NS], F32R, name=f"xsT{k}") for k in range(KC)]
        for it in range(NIT):
            x_nat = xpool.tile([P, D], F32, tag="x_nat")
            nc.sync.dma_start(x_nat, x_d[ts(it, P), :])
            xs_nat = xpool.tile([P, D], F32, tag="xs_nat")
            nc.vector.tensor_mul(out=xs_nat, in0=x_nat, in1=s_rep)
            sq = xpool.tile([P, D], F32, tag="sq")
            nc.scalar.activation(
                sq, xs_nat, AF.Square, accum_out=xn_all[:, it : it + 1]
            )
            for k in range(KC):
                pt = tp_ps.tile([P, P], F32, tag="tp")
                nc.tensor.transpose(pt, xs_nat[:, ts(k, P)], ident)
                nc.vector.tensor_copy(xsT[k][:, ts(it, P)], pt)

        # --- main loop over j-chunks ---
        for jc in range(NJ):
            ysT = [
                ytp.tile([P, JBLK], F32R, tag=f"ysT{k}", name=f"ysT{k}_{jc}")
                for k in range(KC)
            ]
            for sub in range(JSUB):
                y_nat = ypool.tile([P, D], F32, tag="y_nat")
                nc.sync.dma_start(y_nat, y_d[ds(jc * JBLK + sub * P, P), :])
                for k in range(KC):
                    pt = tp_ps.tile([P, P], F32, tag="tp")
                    nc.tensor.transpose(pt, y_nat[:, ts(k, P)], ident)
                    # scale by s (D now on partitions) during PSUM->SBUF copyback
                    nc.vector.tensor_scalar_mul(
                        ysT[k][:, ts(sub, P)], pt, s_part[:, k : k + 1]
                    )

            # yn replicated across all partitions: ones^T @ (ysT^2)
            ps_yn = yn_ps.tile([P, JBLK], F32, tag="yn")
            for k in range(KC):
                ysq = ytp.tile([P, JBLK], F32R, tag=f"ysq{k}")
                nc.scalar.activation(ysq, ysT[k].bitcast(F32), AF.Square)
                nc.tensor.matmul(
                    ps_yn,
                    ones_r,
                    ysq,
                    start=(k == 0),
                    stop=(k == KC - 1),
                )
            yn_rep = ytp.tile([P, JBLK], F32, tag="yn_rep")
            nc.vector.tensor_copy(yn_rep, ps_yn)

            for it in range(NIT):
                po = mm_ps.tile([P, JBLK], F32, tag="mm")
                for k in range(KC):
                    nc.tensor.matmul(
                        po,
                        xsT[k][:, ts(it, P)],
                        ysT[k],
                        start=(k == 0),
                        stop=(k == KC - 1),
                    )
                o_sb = opool.tile([P, JBLK], F32, tag="o")
                # o = -2*cross + xn[i]
                nc.scalar.activation(
                    o_sb, po, AF.Identity, bias=xn_all[:, it : it + 1], scale=-2.0
                )
                # o += yn[j]
                nc.vector.tensor_add(out=o_sb, in0=o_sb, in1=yn_rep)
                nc.sync.dma_start(out_d[ts(it, P), ds(jc * JBLK, JBLK)], o_sb)

    nc.compile()
    return nc


_PROGRAM = None


def _program():
    global _PROGRAM
    if _PROGRAM is None:
        _PROGRAM = _build_program()
    return _PROGRAM


def kernel(x, y, log_scale, **_):
    x = np.ascontiguousarray(x, dtype=np.float32)
    y = np.ascontiguousarray(y, dtype=np.float32)
    log_scale = np.ascontiguousarray(log_scale, dtype=np.float32)

    nc = _program()
    in_maps = [
        {"x": x[c * NS : (c + 1) * NS], "y": y, "log_scale": log_scale}
        for c in range(NCORES)
    ]
    res = run_bass_kernel_spmd(nc, in_maps, list(range(NCORES)))
    return np.concatenate([r["out"] for r in res.results], axis=0)


# revision 7
# speedup vs baseline: 1.2196x; 1.2196x over previous
"""Diagonal Mahalanobis distance kernel for Trainium2 (8 NeuronCores, SPMD).

d2[n, m] = sum_d (s_d * (x[n,d] - y[m,d]))^2
         = ||xs_n||^2 + ||ys_m||^2 - 2 * xs @ ys^T,   xs = x*s, ys = y*s, s = exp(log_scale)

Sharding: x rows (and output rows) split across 8 cores; y and log_scale replicated.
Per core: x_shard (1024, 512) -> out_shard (1024, 8192).

Per-core pipeline (all GEMM work in float32r: 1 cyc/row on the PE vs 4 for fp32):
  - s_part [128, 4]: exp(log_scale), D on partitions; s_rep [128, 512]: replicated
    across partitions via a ones-matmul.
  - x phase: xs = x * s_rep (DVE); xn via ACT Square+accum_out; xsT via PE
    transposes grouped 4-to-a-PSUM-bank, plain DVE copyback (f32r).
  - per 1024-col j-chunk: PE-transpose y into ysT[k] (scale by s_part folded into
    the PSUM->SBUF copyback); yn = ones^T @ (ysT^2) replicated on all partitions;
    GEMM xsT.T @ ysT accumulated over 4 k-chunks into [128,512] PSUM tiles;
    epilogue ACT (-2*psum + xn[i]) then DVE (+ yn_rep), DMA out.
"""

import os
from contextlib import ExitStack

import numpy as np

import concourse.bass as bass
import concourse.tile as tile
from concourse import bacc, mybir
from concourse.bass import ds, ts
from concourse.bass_utils import run_bass_kernel_spmd
from concourse.masks import make_identity

N, M, D = 8192, 8192, 512
NCORES = 8
NS = N // NCORES  # 1024 x-rows per core
P = 128
KC = D // P       # 4 contraction chunks of 128
NIT = NS // P     # 8 i-tiles per core
JBLK = 1024
NJ = M // JBLK    # 8 j-chunks
NG = JBLK // 512  # transpose groups per chunk (4 subtiles each)
NH = JBLK // 512  # psum halves per chunk

F32 = mybir.dt.float32
F32R = mybir.dt.float32r
AF = mybir.ActivationFunctionType


def _build_program():
    nc = bacc.Bacc("TRN2", target_bir_lowering=False, debug=False)

    x_d = nc.dram_tensor("x", [NS, D], F32, kind="ExternalInput").ap()
    y_d = nc.dram_tensor("y", [M, D], F32, kind="ExternalInput").ap()
    ls_d = nc.dram_tensor("log_scale", [D], F32, kind="ExternalInput").ap()
    out_d = nc.dram_tensor("out", [NS, M], F32, kind="ExternalOutput").ap()

    with tile.TileContext(nc) as tc, ExitStack() as ctx:
        consts = ctx.enter_context(tc.tile_pool(name="consts", bufs=1))
        xpool = ctx.enter_context(tc.tile_pool(name="xpool", bufs=5))
        ypool = ctx.enter_context(tc.tile_pool(name="ypool", bufs=6))
        ytp = ctx.enter_context(tc.tile_pool(name="ytp", bufs=2))
        opool = ctx.enter_context(tc.tile_pool(name="opool", bufs=8))
        tp_ps = ctx.enter_context(tc.tile_pool(name="tp_ps", bufs=3, space="PSUM"))
        mm_ps = ctx.enter_context(tc.tile_pool(name="mm_ps", bufs=4, space="PSUM"))

        ident = consts.tile([P, P], F32)
        make_identity(nc, ident)
        ones = consts.tile([P, P], F32)
        nc.vector.memset(ones, 1.0)
        ones_r = consts.tile([P, P], F32R)
        nc.vector.tensor_copy(ones_r, ones)

        # --- scale vectors ---
        ls_part = consts.tile([P, KC], F32)
        nc.sync.dma_start(ls_part, ls_d.rearrange("(o p) -> p o", p=P))
        s_part = consts.tile([P, KC], F32)
        nc.scalar.activation(s_part, ls_part, AF.Exp)

        ls_row = consts.tile([1, D], F32)
        nc.sync.dma_start(ls_row, ls_d[None, :])
        s_row = consts.tile([1, D], F32)
        nc.scalar.activation(s_row, ls_row, AF.Exp)
        ps_srep = mm_ps.tile([P, D], F32, tag="mm")
        nc.tensor.matmul(ps_srep, ones[0:1, :], s_row, start=True, stop=True)
        s_rep = consts.tile([P, D], F32)
        nc.vector.tensor_copy(s_rep, ps_srep)

        # --- x phase: xs, xn, xsT (f32r) ---
        xn_all = consts.tile([P, NIT], F32)
        xsT = [consts.tile([P, NS], F32R, name=f"xsT{k}") for k in range(KC)]
        for itg in range(NIT // 4):
            xs_nats = []
            for it4 in range(4):
                it = itg * 4 + it4
                x_nat = xpool.tile([P, D], F32, tag="x_nat")
                nc.sync.dma_start(x_nat, x_d[ts(it, P), :])
                xs_nat = xpool.tile([P, D], F32, tag="xs_nat")
                nc.vector.tensor_mul(out=xs_nat, in0=x_nat, in1=s_rep)
                sq = xpool.tile([P, D], F32, tag="sq")
                nc.scalar.activation(
                    sq, xs_nat, AF.Square, accum_out=xn_all[:, it : it + 1]
                )
                xs_nats.append(xs_nat)
            for k in range(KC):
                pt = tp_ps.tile([P, 512], F32, tag="tp")
                for it4 in range(4):
                    nc.tensor.transpose(
                        pt[:, ts(it4, P)], xs_nats[it4][:, ts(k, P)], ident
                    )
                nc.vector.tensor_copy(xsT[k][:, ds(itg * 512, 512)], pt)

        # --- main loop over j-chunks ---
        for jc in range(NJ):
            ysT = [
                ytp.tile([P, JBLK], F32R, tag=f"ysT{k}", name=f"ysT{k}_{jc}")
                for k in range(KC)
            ]
            for g in range(NG):
                y_nats = []
                for s4 in range(4):
                    y_nat = ypool.tile([P, D], F32, tag="y_nat")
                    nc.sync.dma_start(
                        y_nat, y_d[ds(jc * JBLK + g * 512 + s4 * P, P), :]
                    )
                    y_nats.append(y_nat)
                for k in range(KC):
                    pt = tp_ps.tile([P, 512], F32, tag="tp")
                    for s4 in range(4):
                        nc.tensor.transpose(
                            pt[:, ts(s4, P)], y_nats[s4][:, ts(k, P)], ident
                        )
                    # scale by s (D on partitions) during copyback, round to f32r
                    nc.vector.tensor_scalar_mul(
                        ysT[k][:, ds(g * 512, 512)], pt, s_part[:, k : k + 1]
                    )

            # yn replicated across partitions: ones^T @ (ysT^2)
            yn_rep = ytp.tile([P, JBLK], F32, tag="yn_rep", name=f"yn_rep_{jc}")
            ysq = [
                ytp.tile([P, JBLK], F32R, tag=f"ysq{k}", name=f"ysq{k}_{jc}")
                for k in range(KC)
            ]
            for k in range(KC):
                nc.scalar.activation(ysq[k], ysT[k].bitcast(F32), AF.Square)
            for h in range(NH):
                ps_yn = mm_ps.tile([P, 512], F32, tag="mm", name=f"psyn{jc}_{h}")
                for k in range(KC):
                    nc.tensor.matmul(
                        ps_yn,
                        ones_r,
                        ysq[k][:, ds(h * 512, 512)],
                        start=(k == 0),
                        stop=(k == KC - 1),
                    )
                nc.vector.tensor_copy(yn_rep[:, ds(h * 512, 512)], ps_yn)

            # GEMM + epilogue
            for it in range(NIT):
                pos = [
                    mm_ps.tile([P, 512], F32, tag="mm", name=f"po{jc}_{it}_{h}")
                    for h in range(NH)
                ]
                for k in range(KC):
                    for h in range(NH):
                        nc.tensor.matmul(
                            pos[h],
                            xsT[k][:, ts(it, P)],
                            ysT[k][:, ds(h * 512, 512)],
                            start=(k == 0),
                            stop=(k == KC - 1),
                        )
                for h in range(NH):
                    o_sb = opool.tile([P, 512], F32, tag="o")
                    nc.scalar.activation(
                        o_sb,
                        pos[h],
                        AF.Identity,
                        bias=xn_all[:, it : it + 1],
                        scale=-2.0,
                    )
                    nc.vector.tensor_add(
                        out=o_sb, in0=o_sb, in1=yn_rep[:, ds(h * 512, 512)]
                    )
                    nc.sync.dma_start(
                        out_d[ts(it, P), ds(jc * JBLK + h * 512, 512)], o_sb
                    )

    nc.compile()
    return nc


_PROGRAM = None


def _program():
    global _PROGRAM
    if _PROGRAM is None:
        _PROGRAM = _build_program()
    return _PROGRAM


def kernel(x, y, log_scale, **_):
    x = np.ascontiguousarray(x, dtype=np.float32)
    y = np.ascontiguousarray(y, dtype=np.float32)
    log_scale = np.ascontiguousarray(log_scale, dtype=np.float32)

    nc = _program()
    in_maps = [
        {"x": x[c * NS : (c + 1) * NS], "y": y, "log_scale": log_scale}
        for c in range(NCORES)
    ]
    res = run_bass_kernel_spmd(nc, in_maps, list(range(NCORES)))
    return np.concatenate([r["out"] for r in res.results], axis=0)


# revision 9
# speedup vs baseline: 1.5436x; 1.2657x over previous
"""Diagonal Mahalanobis distance kernel for Trainium2 (8 NeuronCores, SPMD).

d2[n, m] = sum_d (s_d * (x[n,d] - y[m,d]))^2
         = ||xs_n||^2 + ||ys_m||^2 - 2 * xs @ ys^T,   xs = x*s, ys = y*s, s = exp(log_scale)

Sharding: x rows (and output rows) split across 8 cores; y and log_scale replicated.
Per core: x_shard (1024, 512) -> out_shard (1024, 8192).

The GEMM contracts over D, which must sit on SBUF partitions for both operands.
Instead of PE-transposing on device (expensive: ~100us/core), kernel() passes
host-pre-transposed copies xt = x.T and yt = y.T as extra inputs, so both
operands DMA straight into the right layout. The s^2 scale is folded onto the
x side (cross = (s^2 x) . y^T), so raw y.T feeds the GEMM directly; the row
norms use:
  xn: ACT Square+accum over the natural x tiles (scaled by s_rep),
  yn: W^T @ (yt^2) where W[d, :] = s_d^2 -- an s^2-weighted ones-matmul that
      lands yn replicated across partitions, j on the free axis.
GEMM runs in float32r (1 cyc/row on the PE vs 4 for fp32).
Epilogue per [128,512] tile: ACT (-2*psum + xn[i]) then DVE (+ yn), DMA out.
"""

import os
from contextlib import ExitStack

import numpy as np

import concourse.bass as bass
import concourse.tile as tile
from concourse import bacc, mybir
from concourse.bass import ds, ts
from concourse.bass_utils import run_bass_kernel_spmd

N, M, D = 8192, 8192, 512
NCORES = 8
NS = N // NCORES  # 1024 x-rows per core
P = 128
KC = D // P       # 4 contraction chunks of 128
NIT = NS // P     # 8 i-tiles per core
JBLK = 1024
NJ = M // JBLK    # 8 j-chunks
NH = JBLK // 512  # psum halves per chunk

F32 = mybir.dt.float32
F32R = mybir.dt.float32r
AF = mybir.ActivationFunctionType


def _build_program():
    nc = bacc.Bacc("TRN2", target_bir_lowering=False, debug=False)

    x_d = nc.dram_tensor("x", [NS, D], F32, kind="ExternalInput").ap()
    xt_d = nc.dram_tensor("xt", [D, NS], F32, kind="ExternalInput").ap()
    yt_d = nc.dram_tensor("yt", [D, M], F32R, kind="ExternalInput").ap()
    ls_d = nc.dram_tensor("log_scale", [D], F32, kind="ExternalInput").ap()
    out_d = nc.dram_tensor("out", [NS, M], F32, kind="ExternalOutput").ap()

    with tile.TileContext(nc) as tc, ExitStack() as ctx:
        consts = ctx.enter_context(tc.tile_pool(name="consts", bufs=1))
        xpool = ctx.enter_context(tc.tile_pool(name="xpool", bufs=3))
        ytp = ctx.enter_context(tc.tile_pool(name="ytp", bufs=2))
        opool = ctx.enter_context(tc.tile_pool(name="opool", bufs=8))
        mm_ps = ctx.enter_context(tc.tile_pool(name="mm_ps", bufs=6, space="PSUM"))

        ones = consts.tile([P, P], F32)
        nc.vector.memset(ones, 1.0)

        # --- scale vectors ---
        # s_part[p, k] = exp(log_scale[k*128 + p]) (D on partitions)
        ls_part = consts.tile([P, KC], F32)
        nc.sync.dma_start(ls_part, ls_d.rearrange("(o p) -> p o", p=P))
        s_part = consts.tile([P, KC], F32)
        nc.scalar.activation(s_part, ls_part, AF.Exp)
        s2_part = consts.tile([P, KC], F32)
        nc.vector.tensor_mul(out=s2_part, in0=s_part, in1=s_part)

        # W[k][d, :] = s2[k*128+d] -- weighted-ones lhsT for the yn reduction
        W = [consts.tile([P, P], F32R, name=f"W{k}") for k in range(KC)]
        for k in range(KC):
            nc.vector.tensor_scalar_mul(W[k], ones, s2_part[:, k : k + 1])

        # s_rep[p, d] = exp(log_scale[d]) on every partition (for natural-x scaling)
        ls_row = consts.tile([1, D], F32)
        nc.sync.dma_start(ls_row, ls_d[None, :])
        s_row = consts.tile([1, D], F32)
        nc.scalar.activation(s_row, ls_row, AF.Exp)
        ps_srep = mm_ps.tile([P, D], F32, tag="mm")
        nc.tensor.matmul(ps_srep, ones[0:1, :], s_row, start=True, stop=True)
        s_rep = consts.tile([P, D], F32)
        nc.vector.tensor_copy(s_rep, ps_srep)

        # --- x phase ---
        # xsT2[k] = s^2 * x^T chunk (f32r), the GEMM lhsT
        xsT2 = [consts.tile([P, NS], F32R, name=f"xsT2_{k}") for k in range(KC)]
        for k in range(KC):
            xt_stage = xpool.tile([P, NS], F32, tag="xt_stage")
            nc.sync.dma_start(xt_stage, xt_d[ts(k, P), :])
            nc.vector.tensor_scalar_mul(xsT2[k], xt_stage, s2_part[:, k : k + 1])

        # xn[i] per-partition bias, from natural x tiles
        xn_all = consts.tile([P, NIT], F32)
        for it in range(NIT):
            x_nat = xpool.tile([P, D], F32, tag="x_nat")
            nc.sync.dma_start(x_nat, x_d[ts(it, P), :])
            xs_nat = xpool.tile([P, D], F32, tag="xs_nat")
            nc.vector.tensor_mul(out=xs_nat, in0=x_nat, in1=s_rep)
            sq = xpool.tile([P, D], F32, tag="sq")
            nc.scalar.activation(
                sq, xs_nat, AF.Square, accum_out=xn_all[:, it : it + 1]
            )

        # --- main loop over j-chunks ---
        for jc in range(NJ):
            # raw y^T chunk, straight from DRAM into the f32r GEMM operand
            ysT = [
                ytp.tile([P, JBLK], F32R, tag=f"ysT{k}", name=f"ysT{k}_{jc}")
                for k in range(KC)
            ]
            for k in range(KC):
                nc.sync.dma_start(ysT[k], yt_d[ts(k, P), ds(jc * JBLK, JBLK)])

            # yn (s^2-weighted column sums of yt^2), replicated across partitions
            yn_rep = ytp.tile([P, JBLK], F32, tag="yn_rep", name=f"yn_rep_{jc}")
            ysq = [
                ytp.tile([P, JBLK], F32R, tag=f"ysq{k}", name=f"ysq{k}_{jc}")
                for k in range(KC)
            ]
            for k in range(KC):
                nc.scalar.activation(ysq[k], ysT[k].bitcast(F32), AF.Square)
            for h in range(NH):
                ps_yn = mm_ps.tile([P, 512], F32, tag="mm", name=f"psyn{jc}_{h}")
                for k in range(KC):
                    nc.tensor.matmul(
                        ps_yn,
                        W[k],
                        ysq[k][:, ds(h * 512, 512)],
                        start=(k == 0),
                        stop=(k == KC - 1),
                    )
                nc.vector.tensor_copy(yn_rep[:, ds(h * 512, 512)], ps_yn)

            # GEMM + epilogue
            for it in range(NIT):
                pos = [
                    mm_ps.tile([P, 512], F32, tag="mm", name=f"po{jc}_{it}_{h}")
                    for h in range(NH)
                ]
                for k in range(KC):
                    for h in range(NH):
                        nc.tensor.matmul(
                            pos[h],
                            xsT2[k][:, ts(it, P)],
                            ysT[k][:, ds(h * 512, 512)],
                            start=(k == 0),
                            stop=(k == KC - 1),
                        )
                for h in range(NH):
                    o_sb = opool.tile([P, 512], F32, tag="o")
                    nc.scalar.activation(
                        o_sb,
                        pos[h],
                        AF.Identity,
                        bias=xn_all[:, it : it + 1],
                        scale=-2.0,
                    )
                    nc.vector.tensor_add(
                        out=o_sb, in0=o_sb, in1=yn_rep[:, ds(h * 512, 512)]
                    )
                    nc.sync.dma_start(
                        out_d[ts(it, P), ds(jc * JBLK + h * 512, 512)], o_sb
                    )

    nc.compile()
    return nc


_PROGRAM = None


def _program():
    global _PROGRAM
    if _PROGRAM is None:
        _PROGRAM = _build_program()
    return _PROGRAM


def make_in_maps(x, y, log_scale):
    x = np.ascontiguousarray(x, dtype=np.float32)
    y = np.ascontiguousarray(y, dtype=np.float32)
    log_scale = np.ascontiguousarray(log_scale, dtype=np.float32)

    xt = np.ascontiguousarray(x.T)  # (D, N)
    yt = np.ascontiguousarray(y.T)  # (D, M)

    return [
        {
            "x": x[c * NS : (c + 1) * NS],
            "xt": np.ascontiguousarray(xt[:, c * NS : (c + 1) * NS]),
            "yt": yt,
            "log_scale": log_scale,
        }
        for c in range(NCORES)
    ]


def kernel(x, y, log_scale, **_):
    nc = _program()
    in_maps = make_in_maps(x, y, log_scale)
    res = run_bass_kernel_spmd(nc, in_maps, list(range(NCORES)))
    return np.concatenate([r["out"] for r in res.results], axis=0)


# revision 11
# speedup vs baseline: 1.7728x; 1.1485x over previous
"""Diagonal Mahalanobis distance kernel for Trainium2 (8 NeuronCores, SPMD).

d2[n, m] = sum_d (s_d * (x[n,d] - y[m,d]))^2
         = ||xs_n||^2 + ||ys_m||^2 - 2 * xs @ ys^T,   xs = x*s, ys = y*s, s = exp(log_scale)

Sharding: 4x2 grid — x rows split 4 ways, y rows (output cols) split 2 ways.
Core c = (a, b): x rows [a*2048, (a+1)*2048), y rows [b*4096, (b+1)*4096).
Each core computes a (2048, 4096) block of the distance matrix. This minimizes
HBM reads per core (2KB * (8192/4 + 8192/2) = 12.6MB) vs 1-D sharding (18.8MB);
the kernel is DMA-bound, writes (32MB/core) dominating.

The GEMM contracts over D, which must sit on SBUF partitions for both operands,
so kernel() passes host-pre-transposed xt = x.T and yt = y.T slices — no
on-device transposes of the big operands. The s^2 scale is folded onto the x
side (cross = (s^2 x) . y^T), so raw y.T feeds the GEMM straight from DMA.
Norms:
  yn: W^T @ (yt^2), W[d, :] = s_d^2 — lands yn replicated across partitions,
      j on the free axis (ready for the DVE epilogue add).
  xn: same W-matmul on xt^2 (i on free), then per-128 PE transpose blocks to
      flip i onto partitions for the ACT epilogue bias.
GEMM runs in float32r (1 cyc/row on the PE vs 4 for fp32).
Epilogue per (it, chunk): 2x ACT (-2*psum + xn[i]) into a [128,1024] tile,
DVE (+ yn), one 512KB DMA out (4KB contiguous runs per row).
"""

import os
from contextlib import ExitStack

import numpy as np

import concourse.bass as bass
import concourse.tile as tile
from concourse import bacc, mybir
from concourse.bass import ds, ts
from concourse.bass_utils import run_bass_kernel_spmd
from concourse.masks import make_identity

N, M, D = 8192, 8192, 512
NCORES = 8
GX, GY = 4, 2
RS = N // GX      # 2048 x-rows per core
MS = M // GY      # 4096 y-rows (output cols) per core
P = 128
KC = D // P       # 4 contraction chunks of 128
NIT = RS // P     # 16 i-tiles per core
JBLK = 1024
NJ = MS // JBLK   # 4 j-chunks
NH = JBLK // 512  # psum halves per chunk

F32 = mybir.dt.float32
F32R = mybir.dt.float32r
AF = mybir.ActivationFunctionType


def _build_program():
    nc = bacc.Bacc("TRN2", target_bir_lowering=False, debug=False)

    xt_d = nc.dram_tensor("xt", [D, RS], F32, kind="ExternalInput").ap()
    yt_d = nc.dram_tensor("yt", [D, MS], F32R, kind="ExternalInput").ap()
    ls_d = nc.dram_tensor("log_scale", [D], F32, kind="ExternalInput").ap()
    out_d = nc.dram_tensor("out", [RS, MS], F32, kind="ExternalOutput").ap()

    with tile.TileContext(nc) as tc, ExitStack() as ctx:
        consts = ctx.enter_context(tc.tile_pool(name="consts", bufs=1))
        xpool = ctx.enter_context(tc.tile_pool(name="xpool", bufs=2))
        ytp = ctx.enter_context(tc.tile_pool(name="ytp", bufs=2))
        opool = ctx.enter_context(tc.tile_pool(name="opool", bufs=6))
        mm_ps = ctx.enter_context(tc.tile_pool(name="mm_ps", bufs=5, space="PSUM"))
        tp_ps = ctx.enter_context(tc.tile_pool(name="tp_ps", bufs=2, space="PSUM"))

        ones = consts.tile([P, P], F32)
        nc.vector.memset(ones, 1.0)
        ident = consts.tile([P, P], F32)
        make_identity(nc, ident)

        # --- scales: s2_part[p, k] = exp(2*log_scale[k*128+p]) ---
        ls_part = consts.tile([P, KC], F32)
        nc.sync.dma_start(ls_part, ls_d.rearrange("(o p) -> p o", p=P))
        s2_part = consts.tile([P, KC], F32)
        # exp(2*ls) = s^2 in one ACT op
        nc.scalar.activation(s2_part, ls_part, AF.Exp, scale=2.0)

        # W[k][d, :] = s2[k*128+d] — weighted-ones lhsT for norm reductions
        W = [consts.tile([P, P], F32R, name=f"W{k}") for k in range(KC)]
        for k in range(KC):
            nc.vector.tensor_scalar_mul(W[k], ones, s2_part[:, k : k + 1])

        # --- x phase: xsT2 = s^2 * x^T (f32r GEMM lhsT), xn from xt^2 ---
        xsT2 = [consts.tile([P, RS], F32R, name=f"xsT2_{k}") for k in range(KC)]
        xn_ps = [
            mm_ps.tile([P, 512], F32, tag="mm", name=f"xnps{q}")
            for q in range(RS // 512)
        ]
        for k in range(KC):
            xt_stage = xpool.tile([P, RS], F32, tag="xt_stage")
            nc.sync.dma_start(xt_stage, xt_d[ts(k, P), :])
            nc.vector.tensor_scalar_mul(xsT2[k], xt_stage, s2_part[:, k : k + 1])
            xsq = xpool.tile([P, RS], F32R, tag="xsq")
            nc.scalar.activation(xsq, xt_stage, AF.Square)
            for q in range(RS // 512):
                nc.tensor.matmul(
                    xn_ps[q],
                    W[k],
                    xsq[:, ds(q * 512, 512)],
                    start=(k == 0),
                    stop=(k == KC - 1),
                )
        # flip xn from (i on free) to (i on partitions): per-128 PE transposes
        xn_all = consts.tile([P, NIT], F32)
        xn_rep = consts.tile([P, RS], F32)
        for q in range(RS // 512):
            nc.vector.tensor_copy(xn_rep[:, ds(q * 512, 512)], xn_ps[q])
        for t in range(NIT):
            ptx = tp_ps.tile([P, P], F32, tag="tpx")
            nc.tensor.transpose(ptx, xn_rep[:, ts(t, P)], ident)
            nc.vector.tensor_copy(xn_all[:, t : t + 1], ptx[:, 0:1])

        # --- main loop over j-chunks ---
        for jc in range(NJ):
            # raw y^T chunk, straight from DRAM into the f32r GEMM operand
            ysT = [
                ytp.tile([P, JBLK], F32R, tag=f"ysT{k}", name=f"ysT{k}_{jc}")
                for k in range(KC)
            ]
            for k in range(KC):
                nc.sync.dma_start(ysT[k], yt_d[ts(k, P), ds(jc * JBLK, JBLK)])

            # yn = s^2-weighted column sums of yt^2, replicated across partitions
            yn_rep = ytp.tile([P, JBLK], F32, tag="yn_rep", name=f"yn_rep_{jc}")
            ysq = [
                ytp.tile([P, JBLK], F32R, tag=f"ysq{k}", name=f"ysq{k}_{jc}")
                for k in range(KC)
            ]
            for k in range(KC):
                nc.scalar.activation(ysq[k], ysT[k].bitcast(F32), AF.Square)
            for h in range(NH):
                ps_yn = mm_ps.tile([P, 512], F32, tag="mm", name=f"psyn{jc}_{h}")
                for k in range(KC):
                    nc.tensor.matmul(
                        ps_yn,
                        W[k],
                        ysq[k][:, ds(h * 512, 512)],
                        start=(k == 0),
                        stop=(k == KC - 1),
                    )
                nc.vector.tensor_copy(yn_rep[:, ds(h * 512, 512)], ps_yn)

            # GEMM + epilogue
            for it in range(NIT):
                pos = [
                    mm_ps.tile([P, 512], F32, tag="mm", name=f"po{jc}_{it}_{h}")
                    for h in range(NH)
                ]
                for k in range(KC):
                    for h in range(NH):
                        nc.tensor.matmul(
                            pos[h],
                            xsT2[k][:, ts(it, P)],
                            ysT[k][:, ds(h * 512, 512)],
                            start=(k == 0),
                            stop=(k == KC - 1),
                        )
                o_sb = opool.tile([P, JBLK], F32, tag="o")
                for h in range(NH):
                    nc.scalar.activation(
                        o_sb[:, ds(h * 512, 512)],
                        pos[h],
                        AF.Identity,
                        bias=xn_all[:, it : it + 1],
                        scale=-2.0,
                    )
                nc.vector.tensor_add(out=o_sb, in0=o_sb, in1=yn_rep)
                nc.sync.dma_start(out_d[ts(it, P), ds(jc * JBLK, JBLK)], o_sb)

    nc.compile()
    return nc


_PROGRAM = None


def _program():
    global _PROGRAM
    if _PROGRAM is None:
        _PROGRAM = _build_program()
    return _PROGRAM


def make_in_maps(x, y, log_scale):
    x = np.ascontiguousarray(x, dtype=np.float32)
    y = np.ascontiguousarray(y, dtype=np.float32)
    log_scale = np.ascontiguousarray(log_scale, dtype=np.float32)

    xt = np.ascontiguousarray(x.T)  # (D, N)
    yt = np.ascontiguousarray(y.T)  # (D, M)

    xt_shards = [
        np.ascontiguousarray(xt[:, a * RS : (a + 1) * RS]) for a in range(GX)
    ]
    yt_shards = [
        np.ascontiguousarray(yt[:, b * MS : (b + 1) * MS]) for b in range(GY)
    ]

    return [
        {
            "xt": xt_shards[c // GY],
            "yt": yt_shards[c % GY],
            "log_scale": log_scale,
        }
        for c in range(NCORES)
    ]


def kernel(x, y, log_scale, **_):
    nc = _program()
    in_maps = make_in_maps(x, y, log_scale)
    res = run_bass_kernel_spmd(nc, in_maps, list(range(NCORES)))
    out = np.empty((N, M), dtype=np.float32)
    for c in range(NCORES):
        a, b = c // GY, c % GY
        out[a * RS : (a + 1) * RS, b * MS : (b + 1) * MS] = res.results[c]["out"]
    return out
